# revision 9
# baseline (speedup 1.0000x reference)
"""Trainium2 Bass kernel for nn_Lookahead (causal-lookahead depthwise conv).

y[t, b, f] = sum_{k=0..20} x[t+k, b, f] * weight[f, k]   (zero tail padding)

Strategy:
  - Shard F=1024 across 8 cores (128 features each). Within a core, process
    4 "quarters" of 32 features, pipelined (load / compute / store overlap).
  - Host pre-lays-out x as [q, i, s, b, f] so DMA loads land directly in a
    time-on-partitions SBUF layout [s=128, (i, b, f)]; the time conv becomes
    a banded-Toeplitz matmul on the TensorEngine:
        out[tau, (i,b)] = sum_s band_f[s, tau] * x[128*i + s, b, f]
    with band_f[s, tau] = w[f, s-tau] for 0 <= s-tau <= 20.
    Rows 0..127 of the band (L1) consume x tile i; rows 128..147 (L2)
    consume the first 20 rows of x tile i+1 via PSUM accumulation.
  - PSUM evacuation alternates VectorE / ScalarE; DMA out in a layout the
    host transposes back.
"""

import sys

sys.path.insert(0, "/opt/trn_rl_repo")

import numpy as np

T, B, F, K = 2048, 16, 1024, 21
CTX = K - 1
NCORES = 8
FC = F // NCORES  # 128 features per core
NQ = 4            # quarters per core
FQ = FC // NQ     # 32 features per quarter
S = 128           # time-tile size (partition dim)
NI = T // S       # 16 time tiles
SB = 148          # band rows: 128 (L1) + 20 (L2)

_MODULE_CACHE = {}


def build_module(repeat=1):
    key = ("nc", repeat)
    if key in _MODULE_CACHE:
        return _MODULE_CACHE[key]
    import concourse.bacc as bacc
    import concourse.mybir as mybir
    from concourse.tile import TileContext

    dt = mybir.dt.float32
    nc = bacc.Bacc("TRN2", target_bir_lowering=False, debug=False,
                   num_devices=NCORES)

    x_d = nc.dram_tensor("x", [NQ, NI, S, B * FQ], dt, kind="ExternalInput")
    b_d = nc.dram_tensor("bands", [NQ, FQ, SB, S], dt, kind="ExternalInput")
    y_d = nc.dram_tensor("y", [NQ, S, NI * B * FQ], dt, kind="ExternalOutput")

    with TileContext(nc) as tc:
        with tc.tile_pool(name="xp", bufs=2) as xp, \
             tc.tile_pool(name="bp", bufs=2) as bp, \
             tc.tile_pool(name="yp", bufs=2) as yp, \
             tc.tile_pool(name="pp", bufs=4, space="PSUM") as pp:
            for q in [q for _ in range(repeat) for q in range(NQ)]:
                xq = xp.tile([S, NI * B * FQ], dt, tag="x")
                b1 = bp.tile([S, FQ * S], dt, tag="b1")
                b2 = bp.tile([CTX, FQ * S], dt, tag="b2")
                ysb = yp.tile([S, NI * B * FQ], dt, tag="y")

                nc.sync.dma_start(
                    out=xq[:],
                    in_=x_d.ap()[q].rearrange("i s m -> s i m"))
                nc.sync.dma_start(
                    out=b1[:],
                    in_=b_d.ap()[q, :, 0:S, :].rearrange("f s t -> s f t"))
                nc.sync.dma_start(
                    out=b2[:],
                    in_=b_d.ap()[q, :, S:SB, :].rearrange("f s t -> s f t"))

                xqr = xq[:].rearrange("s (i b f) -> s i b f", i=NI, b=B, f=FQ)
                b1r = b1[:].rearrange("s (f t) -> s f t", f=FQ, t=S)
                b2r = b2[:].rearrange("s (f t) -> s f t", f=FQ, t=S)
                ysr = ysb[:].rearrange("t (i b f) -> t i b f", i=NI, b=B, f=FQ)

                for fi in range(FQ):
                    pt = pp.tile([S, NI * B], dt, tag="ps")
                    # L1: all 16 time tiles, full 128-row contraction.
                    nc.tensor.matmul(
                        pt[:, :],
                        lhsT=b1r[:, fi, :],
                        rhs=xqr[:, :, :, fi],
                        start=True, stop=False, skip_group_check=True)
                    # L2: 20-row contraction against the *next* time tile;
                    # out tile 15 has no L2 contribution (zero tail pad).
                    nc.tensor.matmul(
                        pt[:, 0:(NI - 1) * B],
                        lhsT=b2r[:, fi, :],
                        rhs=xqr[0:CTX, 1:NI, :, fi],
                        start=False, stop=True, skip_group_check=True)
                    if fi % 2 == 0:
                        nc.vector.tensor_copy(ysr[:, :, :, fi], pt[:, :])
                    else:
                        nc.scalar.copy(ysr[:, :, :, fi], pt[:, :])

                nc.sync.dma_start(out=y_d.ap()[q], in_=ysb[:])

    nc.compile()
    _MODULE_CACHE[key] = nc
    return nc


def prep_x(x):
    """x (2048, 16, 1024) -> per-core [4, 16, 128, 512] arrays."""
    xr = np.asarray(x, dtype=np.float32).reshape(NI, S, B, NCORES, NQ, FQ)
    xt = xr.transpose(3, 4, 0, 1, 2, 5)  # (c, q, i, s, b, f)
    return np.ascontiguousarray(xt).reshape(NCORES, NQ, NI, S, B * FQ)


def prep_bands(weight):
    """weight (1024, 21) -> per-core [4, 32, 148, 128] banded matrices."""
    w = np.asarray(weight, dtype=np.float32).reshape(NCORES, FC, K)
    band = np.zeros((NCORES, FC, SB, S), np.float32)
    tau = np.arange(S)
    for k in range(K):
        band[:, :, tau + k, tau] = w[:, :, k][..., None]
    return band.reshape(NCORES, NQ, FQ, SB, S)


def assemble_y(shards):
    """per-core (4, 128, 16*16*32) -> (2048, 16, 1024)."""
    y = np.stack(shards).reshape(NCORES, NQ, S, NI, B, FQ)  # (c, q, tau, i, b, f)
    y = y.transpose(3, 2, 4, 0, 1, 5)  # (i, tau, b, c, q, f)
    return np.ascontiguousarray(y).reshape(T, B, F)


def kernel(x, weight, tail_padding):
    from concourse.bass_utils import run_bass_kernel_spmd

    nc = build_module()
    xs = prep_x(x)  # noqa
    bs = prep_bands(weight)
    in_maps = [{"x": xs[c], "bands": bs[c]} for c in range(NCORES)]
    res = run_bass_kernel_spmd(nc, in_maps, list(range(NCORES)))
    shards = [res.results[c]["y"] for c in range(NCORES)]
    y = assemble_y(shards)
    seq_len = T if int(np.asarray(tail_padding)) else T - CTX
    return y[:seq_len]
